# revision 1
# baseline (speedup 1.0000x reference)
"""CLCE loss kernel for Trainium2 (8 NeuronCores, SPMD).

Loss = 0.5 * cl + 0.5 * ce where
  cl_i = logsumexp(loss_temp_i) - slot0_i   over a [N, 2N-1] packed row
  ce   = cross-entropy of y_pred vs y_true.

Decomposition (exact, validated in f64 against the reference formula):
  cl_i = log(exp(slot0_i) + (T_i - P_i) + (2N-2 - num_neg_i)) - slot0_i
where
  T_i  = sum_j exp((xn_i . xn_j + 1) * 0.25)      <- the O(N^2 D) part, on device
  P_i  = sum_{j: y_j = y_i} exp(sim_ij)           <- O(N * class_size), on host
  slot0_i = sim_{i, first same-class j != i}      <- O(N), on host
  R_i  = sum_j exp(y_pred_ij)                     <- on device
  ce_i = log(R_i) - y_pred[i, y_i]

Device sharding: core c computes rows [512c, 512(c+1)) of the similarity
matrix as an fp8e4m3 DoubleRow matmul (2 MACs/cell/cycle; embeddings are
pre-scaled by S8 on the host so quantization error stays ~1e-4 relative on
each sim entry, which averages to ~1e-5 on the final scalar loss), with the
exp+row-sum fused into Scalar-engine activations (accum_out).  The
correction terms P_i/slot0_i are computed on the host in full precision
from the same normalized embeddings, so the handful of same-class entries
inside T_i cancel to fp8-noise level.
"""

import os
from contextlib import ExitStack

import numpy as np

import concourse.bass as bass
import concourse.tile as tile
from concourse import bacc, mybir
from concourse.bass_utils import run_bass_kernel_spmd

N, D, C = 4096, 1024, 512
TAU = 0.5
LAMBD = 0.5
NCORES = 8
BLK = N // NCORES          # 512 rows per core
P = 128                    # partitions
KT = D // 256              # 4 DoubleRow contraction super-tiles (256 each)
MT = BLK // P              # 4 output row tiles per core
W = 1024                   # column-chunk width (2 psum banks)
HC = N // W                # 4 column chunks
NS = W // 512              # matmuls per chunk k-step
S8 = 16.0                  # fp8 pre-scale for the embeddings

_F32 = mybir.dt.float32
_FP8 = mybir.dt.float8e4
_EXP = mybir.ActivationFunctionType.Exp
_DR = mybir.MatmulPerfMode.DoubleRow


def _build_kernel(tc, xt, wt, yp, out):
    """Emit the per-core Tile kernel.

    xt:  [KT*P, 2*N]   fp8  row kk*128+p, col i*N+n = S8*xn[n, kk*256+128i+p]
    wt:  [KT*P, 2*BLK] fp8  this core's column block, same packing
    yp:  [P, MT*C]     f32  this core's y_pred block, partition-major packed
    out: [P, MT*HC+MT] f32  T chunk-sums then R row-sums
    """
    nc = tc.nc
    with ExitStack() as ctx:
        pers = ctx.enter_context(tc.tile_pool(name="pers", bufs=1))
        epool = ctx.enter_context(tc.tile_pool(name="epool", bufs=2))
        psum = ctx.enter_context(
            tc.tile_pool(name="psum", bufs=4, space=bass.MemorySpace.PSUM)
        )

        # per-(kk, h) input tiles -> exact DMA->matmul dependencies.
        # The weights and the first column chunk arrive fused in one DMA per
        # kk (WX0) to halve the issue slots pacing the pipeline start.
        WX0 = [
            pers.tile([P, 2, BLK + W], _FP8, name=f"wx0_{k}", tag=f"wx0_{k}")
            for k in range(KT)
        ]
        XT = [
            [None] + [
                pers.tile([P, 2, W], _FP8, name=f"xtt{k}_{h}", tag=f"xtt{k}_{h}")
                for h in range(1, HC)
            ]
            for k in range(KT)
        ]
        WT = [WX0[k][:, :, 0:BLK] for k in range(KT)]
        for k in range(KT):
            XT[k][0] = WX0[k][:, :, BLK:BLK + W]
        YPB = pers.tile([P, MT * C], _F32)     # 8 KiB/partition
        # out layout: [Tparts (MT*HC) | Rparts (MT)]
        OUTSB = pers.tile([P, MT * HC + MT], _F32)
        bias_s = pers.tile([P, 1], _F32)       # 0.5*TAU for the sim affine
        bias_z = pers.tile([P, 1], _F32)       # 0.0 for plain exp
        warm = pers.tile([P, 1], _F32)

        ZW = pers.tile([P, 512], mybir.dt.bfloat16)  # zeros, PE warm-up operand

        nc.gpsimd.memset(ZW[:], 0.0)
        nc.gpsimd.memset(bias_s[:], 0.5 * TAU)
        nc.gpsimd.memset(bias_z[:], 0.0)
        # warm the exp table (ACT_TABLE_LOAD ~2.7us) before any data lands
        nc.scalar.activation(warm[:], bias_z[:], _EXP, bias=bias_z[:], scale=1.0)

        # PE warm-up: dummy matmuls spanning the input-DMA latency (~7us)
        # flip the HAM clock gate to 8/8 so the real stream starts at 2.4GHz
        wps = psum.tile([P, W], _F32, tag="ps")
        for _ in range(12):
            nc.tensor.matmul(wps[:, 0:512], ZW[:, 0:P], ZW[:], start=True, stop=True)

        # --- input DMAs.  Sync HWDGE carries the matmul operands in exactly
        # the order the PE consumes them: (WT kk, XT[kk][0]) pairs pace the
        # first chunk, then the later column chunks.  y_pred rides the
        # scalar HWDGE queue so it neither delays the sync stream nor the
        # CE activations. ---
        nc.scalar.dma_start(YPB[:], yp[:])
        xt3 = xt.rearrange("r (i n) -> r i n", i=2)
        wt3 = wt.rearrange("r (i n) -> r i n", i=2)
        for k in range(KT):
            nc.sync.dma_start(WX0[k][:], wt3[k * P:(k + 1) * P, :, :])
        for h in range(1, HC):
            for k in range(KT):
                nc.sync.dma_start(
                    XT[k][h][:],
                    xt3[k * P:(k + 1) * P, :, h * W:(h + 1) * W],
                )

        # --- CE: R[p, t] = sum_c exp(y_pred) ---
        for t in range(MT):
            et = epool.tile([P, W], _F32)
            nc.scalar.activation(
                et[:, 0:C], YPB[:, t * C:(t + 1) * C], _EXP,
                bias=bias_z[:], scale=1.0,
                accum_out=OUTSB[:, MT * HC + t:MT * HC + t + 1],
            )

        # --- main: sim block matmul + fused exp/row-sum ---
        # dot_scaled = S8^2 * xn_i . xn_j ; sim = (dot + 1) * 0.5 * TAU
        # -> exp(scale * dot_scaled + bias), scale = 0.5*TAU/S8^2, bias = 0.25
        act_scale = 0.5 * TAU / (S8 * S8)

        # first column chunk: k-outer over m=0..2 so the PE does three
        # m-tiles' work per arriving (WT k, XT k) pair -- stays dense behind
        # the DMA stream instead of stalling per k (which would re-throttle
        # the clock gate).  m=3 runs as a pipelined chunk afterward so its
        # matmuls cover the m=0..2 exp/row-sum drain and h=1 starts with a
        # free psum slot.
        ps_h0 = [
            psum.tile([P, W], _F32, tag="ps", name=f"psh0_{m}")
            for m in range(MT - 1)
        ]
        for k in range(KT):
            for m in range(MT - 1):
                for ns in range(NS):
                    nc.tensor.matmul(
                        ps_h0[m][:, ns * 512:(ns + 1) * 512],
                        WT[k][:, :, m * P:(m + 1) * P],
                        XT[k][0][:, :, ns * 512:(ns + 1) * 512],
                        start=(k == 0),
                        stop=(k == KT - 1),
                        perf_mode=_DR,
                    )
        for m in range(MT - 1):
            et = epool.tile([P, W], _F32)
            nc.scalar.activation(
                et[:], ps_h0[m][:], _EXP,
                bias=bias_s[:], scale=act_scale,
                accum_out=OUTSB[:, m * HC:m * HC + 1],
            )

        # remaining chunks: m-outer with psum-pool ping-pong (zero steady
        # state PE stalls; exp+row-sum runs concurrently on ScalarE)
        for h, m in [(0, MT - 1)] + [
            (h, m) for h in range(1, HC) for m in range(MT)
        ]:
            if True:
                ps = psum.tile([P, W], _F32, tag="ps")
                for k in range(KT):
                    for ns in range(NS):
                        nc.tensor.matmul(
                            ps[:, ns * 512:(ns + 1) * 512],
                            WT[k][:, :, m * P:(m + 1) * P],
                            XT[k][h][:, :, ns * 512:(ns + 1) * 512],
                            start=(k == 0),
                            stop=(k == KT - 1),
                            perf_mode=_DR,
                        )
                et = epool.tile([P, W], _F32)
                nc.scalar.activation(
                    et[:], ps[:], _EXP,
                    bias=bias_s[:], scale=act_scale,
                    accum_out=OUTSB[:, m * HC + h:m * HC + h + 1],
                )

        nc.scalar.dma_start(out[:], OUTSB[:])


_NC_CACHE = None


def _get_nc():
    global _NC_CACHE
    if _NC_CACHE is None:
        nc = bacc.Bacc(
            "TRN2", target_bir_lowering=False, debug=False,
            enable_asserts=False, num_devices=NCORES,
        )
        xt_d = nc.dram_tensor("xt", [KT * P, 2 * N], _FP8, kind="ExternalInput")
        wt_d = nc.dram_tensor(
            "wt", [KT * P, 2 * (BLK + W)], _FP8, kind="ExternalInput"
        )
        yp_d = nc.dram_tensor("yp", [P, MT * C], _F32, kind="ExternalInput")
        out_d = nc.dram_tensor(
            "out", [P, MT * HC + MT], _F32, kind="ExternalOutput"
        )
        with tile.TileContext(nc) as tc:
            _build_kernel(tc, xt_d.ap(), wt_d.ap(), yp_d.ap(), out_d.ap())
        nc.compile()
        _NC_CACHE = nc
    return _NC_CACHE


def _pack_fp8(zT, cols):
    """[D, ncols] f32 -> [KT*P, 2*ncols] fp8 with the DoubleRow pairing
    row kk*128+p, col i*ncols+n  <->  contraction index kk*256 + 128i + p."""
    fp8np = mybir.dt.np(_FP8)
    q = zT.reshape(KT, 2, P, cols).transpose(0, 2, 1, 3).reshape(KT * P, 2 * cols)
    return np.ascontiguousarray(q.astype(fp8np))


def _run_device(xnT, y_pred, trace=False):
    """Run the SPMD kernel; returns (T[N], R[N]) f64 and the raw results."""
    zT = (xnT * S8).astype(np.float32)  # [D, N], pre-scaled
    xt8 = _pack_fp8(zT, N)
    in_maps = []
    for c in range(NCORES):
        blk = slice(c * BLK, (c + 1) * BLK)
        ypb = (
            np.ascontiguousarray(y_pred[blk])
            .reshape(MT, P, C).transpose(1, 0, 2).reshape(P, MT * C)
        )
        wx0 = np.concatenate([zT[:, blk], zT[:, 0:W]], axis=1)
        in_maps.append({
            "xt": xt8,
            "wt": _pack_fp8(np.ascontiguousarray(wx0), BLK + W),
            "yp": np.ascontiguousarray(ypb),
        })
    res = run_bass_kernel_spmd(
        _get_nc(), in_maps, core_ids=list(range(NCORES)), trace=trace,
    )
    T = np.empty(N, np.float64)
    R = np.empty(N, np.float64)
    for c, r in enumerate(res.results):
        o = r["out"].astype(np.float64)  # [128, MT*HC + MT]
        for m in range(MT):
            rows = slice(c * BLK + m * P, c * BLK + (m + 1) * P)
            T[rows] = o[:, m * HC:(m + 1) * HC].sum(axis=1)
            R[rows] = o[:, MT * HC + m]
    return T, R, res


def kernel(layer_embeds, y_true, y_pred):
    x = np.asarray(layer_embeds, dtype=np.float32)
    yt = np.asarray(y_true).astype(np.int64)
    yp = np.asarray(y_pred, dtype=np.float32)

    # normalize rows (torch-style eps clip)
    norms = np.maximum(
        np.sqrt((x.astype(np.float64) ** 2).sum(1, keepdims=True)), 1e-8
    )
    xn = (x / norms).astype(np.float32)
    xnT = np.ascontiguousarray(xn.T)  # [D, N]

    trace = bool(int(os.environ.get("CLCE_TRACE", "0")))
    T, R, res = _run_device(xnT, yp, trace=trace)
    if trace:
        kernel.last_results = res

    # --- host-side small terms (O(N * class_size)) ---
    # P_ must match what the device summed for the same-class entries, i.e.
    # the fp8-quantized sim values, so quantize the same way here.
    fp8np = mybir.dt.np(_FP8)
    xq = (xn * S8).astype(fp8np).astype(np.float64) / S8  # device-visible xn
    counts = np.bincount(yt, minlength=C)
    P_ = np.zeros(N, np.float64)
    slot0 = np.zeros(N, np.float64)
    for cval in np.unique(yt):
        idx = np.where(yt == cval)[0]
        subq = xq[idx]
        sq = (subq @ subq.T + 1.0) * (0.5 * TAU)   # device-matching sim
        P_[idx] = np.exp(sq).sum(1)
        if len(idx) >= 2:
            # slot0 feeds the final formula directly -> use full precision
            sub = xn[idx].astype(np.float64)
            s = (sub @ sub.T + 1.0) * (0.5 * TAU)
            firstpos = np.where(np.arange(len(idx)) == 0, 1, 0)
            slot0[idx] = s[np.arange(len(idx)), firstpos]

    num_neg = N - counts[yt]
    S = T - P_
    Z = (2 * N - 2 - num_neg).astype(np.float64)
    cl = (np.log(np.exp(slot0) + S + Z) - slot0).mean()
    ce = (
        np.log(R) - yp[np.arange(N), yt].astype(np.float64)
    ).mean()
    loss = LAMBD * cl + (1.0 - LAMBD) * ce
    return np.asarray(loss, dtype=np.float32)



# revision 5
# speedup vs baseline: 1.1097x; 1.1097x over previous
"""CLCE loss kernel for Trainium2 (8 NeuronCores, SPMD) — symmetric version.

Loss = 0.5 * cl + 0.5 * ce where
  cl_i = log(exp(slot0_i) + (T_i - P_i) + Z_i) - slot0_i
  T_i  = sum_j exp((xn_i . xn_j + 1) * 0.25)     <- O(N^2 D), on device
  P_i, slot0_i: same-class corrections, on host (exact, tiny)
  ce: cross-entropy of y_pred, on host in f64 (O(N*C), tiny)

Device: the N x N exp-sim row-sum exploits symmetry — only the upper
triangle of the 32x32 grid of 128-cells is computed (528 of 1024 cells).
Each computed strip contributes its row-sums via the Scalar engine's
accum_out, and its mirrored contribution via column-sums: exp tiles are
accumulated per column-tile on the Vector engine (bf16) and reduced
across partitions with one ones-matmul per column slot.

Uniform SPMD structure: every core runs the identical 9-item schedule
(diag staircase upper/lower + 7 off-diagonal 256-row half-blocks) over
6 data slots (4 column tiles + 2 flexible weight-row slots).  The host
chooses per-core slot contents so the 8 cores tile the triangle exactly:
  slots(c)  = [c, c+1, c+2, T3[c]]  (mod 8), T3 = [4,5,6,7,7,4,5,6]
  W halves  = difference-class patches (d2-h1, d3-h0, d3-h1, d4 halves)
Embeddings are fp8 (pre-scaled by S8) with DoubleRow matmuls, identical
quantization to what the host correction terms replicate.
"""

import os
from contextlib import ExitStack

import numpy as np

import concourse.bass as bass
import concourse.tile as tile
from concourse import bacc, mybir
from concourse.bass_utils import run_bass_kernel_spmd

N, D, C = 4096, 1024, 512
TAU = 0.5
LAMBD = 0.5
NCORES = 8
P = 128                    # partitions
KT = D // 256              # 4 DoubleRow contraction super-tiles
TW = 512                   # tile width (columns per tile slot)
S8 = 16.0                  # fp8 pre-scale for the embeddings
NSLOT = 6                  # 4 column tiles + 2 weight-row slots
NWARM = 8                  # PE warm-up matmuls

_F32 = mybir.dt.float32
_BF16 = mybir.dt.bfloat16
_FP8 = mybir.dt.float8e4
_EXP = mybir.ActivationFunctionType.Exp
_DR = mybir.MatmulPerfMode.DoubleRow

# ---------------- cover tables (validated exact) ----------------
T3 = [4, 5, 6, 7, 7, 4, 5, 6]


def _slots_of(c):
    return [c, (c + 1) % 8, (c + 2) % 8, T3[c]]


def _whalves_of(c):
    # (tile, half) for W halves 0..3 (slot4 m01, slot4 m23, slot5 m01, slot5 m23)
    return [((c - 2) % 8, 1), ((c - 2) % 8, 0), ((c - 1) % 8, 1),
            (T3[c] - 4, 0 if c < 4 else 1)]


# item: (row_slot, mlo, col_slot, kind); row_slot 0..3 = tile, 4..5 = W slots
ITEMS = [
    (0, 0, 0, 'DU'),   # I0 diag staircase rows m0,m1
    (0, 2, 0, 'DL'),   # I1 diag staircase rows m2,m3
    (0, 0, 1, 'OFF'),  # I2 (c -> c+1, h0)
    (1, 2, 2, 'OFF'),  # I3 (c+1 -> c+2, h1)
    (0, 0, 2, 'OFF'),  # I4 (c -> c+2, h0)
    (4, 0, 0, 'OFF'),  # I5 W patch, cols slot0
    (4, 2, 1, 'OFF'),  # I6 W patch, cols slot1
    (5, 0, 2, 'OFF'),  # I7 W patch, cols slot2
    (5, 2, 3, 'OFF'),  # I8 W patch, cols slot3
]
# device schedule order (I1 last: its final chunk has no colsum -> short tail)
ORDER = [0, 2, 5, 6, 3, 4, 7, 8, 1]


def _item_chunks(kind, mloc):
    """(mm_lo, mm_hi, [(a, b, to_colsum), ...]) column ranges for one m."""
    if kind == 'OFF':
        return (0, 512, [(0, 512, True)])
    if kind == 'DU':
        if mloc == 0:
            return (0, 512, [(0, 128, False), (128, 512, True)])
        return (128, 512, [(128, 256, False), (256, 512, True)])
    # DL
    if mloc == 0:
        return (256, 512, [(256, 384, False), (384, 512, True)])
    return (384, 512, [(384, 512, False)])


def _item_rows(c, item, mloc):
    """Global start row of the 128-row group (item, mloc) computes."""
    slot, mlo = item[0], item[1]
    if slot < 4:
        return _slots_of(c)[slot] * 512 + (mlo + mloc) * 128
    t, h = _whalves_of(c)[(slot - 4) * 2 + mlo // 2]
    return t * 512 + h * 256 + mloc * 128


def _emission_chunks():
    """(item_idx, mloc, a, b, cs) in device emission order = accum col order."""
    out = []
    for it in ORDER:
        kind = ITEMS[it][3]
        for mloc in (0, 1):
            mm_lo, mm_hi, chunks = _item_chunks(kind, mloc)
            for (a, b, cs) in chunks:
                out.append((it, mloc, a, b, cs))
    return out


NRS = len(_emission_chunks())   # rowsum output columns (21)


# ---------------- device kernel ----------------
def _build_kernel(tc, xt, out1, out2):
    nc = tc.nc
    act_scale = 0.5 * TAU / (S8 * S8)
    with ExitStack() as ctx:
        pers = ctx.enter_context(tc.tile_pool(name="pers", bufs=1))
        epool = ctx.enter_context(tc.tile_pool(name="epool", bufs=3))
        psum = ctx.enter_context(
            tc.tile_pool(name="psum", bufs=4, space=bass.MemorySpace.PSUM)
        )
        cspsum = ctx.enter_context(
            tc.tile_pool(name="cspsum", bufs=2, space=bass.MemorySpace.PSUM)
        )

        SLOT = [
            pers.tile([P, KT, 2, TW], _FP8, name=f"slot{s}", tag=f"slot{s}")
            for s in range(NSLOT)
        ]
        A = [pers.tile([P, TW], _BF16, name=f"acc{s}") for s in range(4)]
        OUTSB = pers.tile([P, NRS], _F32)
        CSSB = pers.tile([1, 4 * TW], _F32)
        bias_s = pers.tile([P, 1], _F32)
        bias_z = pers.tile([P, 1], _F32)
        warm = pers.tile([P, 1], _F32)
        ones = pers.tile([P, 1], _BF16)
        ZW = pers.tile([P, 512], _BF16)

        nc.gpsimd.memset(ZW[:], 0.0)
        nc.gpsimd.memset(bias_s[:], 0.5 * TAU)
        nc.gpsimd.memset(bias_z[:], 0.0)
        nc.gpsimd.memset(ones[:], 1.0)
        for s in range(4):
            nc.gpsimd.memset(A[s][:], 0.0)
        # warm the exp table before any data lands
        nc.scalar.activation(warm[:], bias_z[:], _EXP, bias=bias_z[:], scale=1.0)

        # PE warm-up: dummy matmuls spanning the input-DMA latency flip the
        # HAM clock gate to 8/8 so the real stream runs at 2.4GHz
        wps = psum.tile([P, 512], _F32, tag="ps")
        for _ in range(NWARM):
            nc.tensor.matmul(wps[:, 0:512], ZW[:, 0:P], ZW[:], start=True,
                             stop=True)

        # input DMAs: two HWDGE queues in parallel, ordered by first use
        xt6 = xt.rearrange("p (s k i n) -> p s k i n", s=NSLOT, k=KT, i=2)
        for s in (0, 1, 2):
            nc.sync.dma_start(SLOT[s][:], xt6[:, s])
        for s in (4, 5, 3):
            nc.scalar.dma_start(SLOT[s][:], xt6[:, s])

        # main schedule
        rs_col = 0
        # colsum reduction points (item_idx -> A slots): each A is reduced
        # one item after its last contribution so the ones-matmul never
        # stalls the PE waiting on the DVE add.  A0 gets DL chunk-B parts,
        # so CS0 tails the last item (I1).
        cs_after = {3: [1], 8: [2], 1: [3, 0]}

        def colsum_reduce(s):
            cps = cspsum.tile([1, TW], _F32, tag="cs")
            nc.tensor.matmul(cps[:, :], ones[:, 0:1], A[s][:, :], start=True,
                             stop=True)
            nc.vector.tensor_copy(CSSB[0:1, s * TW:(s + 1) * TW], cps[:, :])

        for it in ORDER:
            row_slot, mlo, col_slot, kind = ITEMS[it]
            for mloc in (0, 1):
                mm_lo, mm_hi, chunks = _item_chunks(kind, mloc)
                mcol = mlo + mloc
                ps = psum.tile([P, 512], _F32, tag="ps")
                for k in range(KT):
                    nc.tensor.matmul(
                        ps[:, mm_lo:mm_hi],
                        SLOT[row_slot][:, k, :, mcol * P:(mcol + 1) * P],
                        SLOT[col_slot][:, k, :, mm_lo:mm_hi],
                        start=(k == 0),
                        stop=(k == KT - 1),
                        perf_mode=_DR,
                    )
                for (a, b, cs) in chunks:
                    et = epool.tile([P, 512], _BF16, tag="et")
                    nc.scalar.activation(
                        et[:, 0:b - a], ps[:, a:b], _EXP,
                        bias=bias_s[:], scale=act_scale,
                        accum_out=OUTSB[:, rs_col:rs_col + 1],
                    )
                    rs_col += 1
                    if cs:
                        nc.vector.tensor_add(
                            A[col_slot][:, a:b], A[col_slot][:, a:b],
                            et[:, 0:b - a],
                        )
            for s in cs_after.get(it, ()):
                colsum_reduce(s)

        nc.scalar.dma_start(out1[:], OUTSB[:])
        nc.sync.dma_start(out2[:], CSSB[:])


_NC_CACHE = None


def _get_nc():
    global _NC_CACHE
    if _NC_CACHE is None:
        nc = bacc.Bacc(
            "TRN2", target_bir_lowering=False, debug=False,
            enable_asserts=False, num_devices=NCORES,
        )
        xt_d = nc.dram_tensor("xt", [P, NSLOT * KT * 2 * TW], _FP8,
                              kind="ExternalInput")
        out1_d = nc.dram_tensor("out1", [P, NRS], _F32, kind="ExternalOutput")
        out2_d = nc.dram_tensor("out2", [1, 4 * TW], _F32,
                                kind="ExternalOutput")
        with tile.TileContext(nc) as tc:
            _build_kernel(tc, xt_d.ap(), out1_d.ap(), out2_d.ap())
        nc.compile()
        _NC_CACHE = nc
    return _NC_CACHE


def _pack_cols(cols):
    """[D, 512] fp8 column block -> [P, 4096] with DoubleRow pairing:
    partition p, byte (k*2 + i)*512 + n  <->  contraction index
    k*256 + 128*i + p  of column n."""
    q = cols.reshape(KT, 2, P, TW).transpose(2, 0, 1, 3).reshape(P, KT * 2 * TW)
    return np.ascontiguousarray(q)


def _run_device(zq8, trace=False):
    """zq8: [D, N] fp8 pre-scaled quantized embeddings (as fp8 np dtype)."""
    in_maps = []
    for c in range(NCORES):
        slots = _slots_of(c)
        wh = _whalves_of(c)
        parts = []
        for s in range(4):
            t = slots[s]
            parts.append(_pack_cols(zq8[:, t * 512:(t + 1) * 512]))
        for w0 in (0, 2):  # slots 4, 5
            blk = np.concatenate(
                [zq8[:, wh[w0 + j][0] * 512 + wh[w0 + j][1] * 256:][:, :256]
                 for j in range(2)], axis=1)
            parts.append(_pack_cols(np.ascontiguousarray(blk)))
        in_maps.append({"xt": np.concatenate(parts, axis=1)})
    res = run_bass_kernel_spmd(
        _get_nc(), in_maps, core_ids=list(range(NCORES)), trace=trace,
    )
    T = np.zeros(N, np.float64)
    chunks = _emission_chunks()
    for c, r in enumerate(res.results):
        o1 = r["out1"].astype(np.float64)   # [P, NRS]
        o2 = r["out2"].astype(np.float64)   # [1, 4*TW]
        slots = _slots_of(c)
        for col, (it, mloc, a, b, cs) in enumerate(chunks):
            r0 = _item_rows(c, ITEMS[it], mloc)
            T[r0:r0 + P] += o1[:, col]
        for s in range(4):
            t = slots[s]
            T[t * 512:(t + 1) * 512] += o2[0, s * TW:(s + 1) * TW]
    return T, res


def kernel(layer_embeds, y_true, y_pred):
    x = np.asarray(layer_embeds, dtype=np.float32)
    yt = np.asarray(y_true).astype(np.int64)
    yp = np.asarray(y_pred, dtype=np.float32)

    # normalize rows (torch-style eps clip)
    norms = np.maximum(
        np.sqrt((x.astype(np.float64) ** 2).sum(1, keepdims=True)), 1e-8
    )
    xn = (x / norms).astype(np.float32)
    fp8np = mybir.dt.np(_FP8)
    zq8 = np.ascontiguousarray((xn.T * S8).astype(np.float32)).astype(fp8np)

    trace = bool(int(os.environ.get("CLCE_TRACE", "0")))
    T, res = _run_device(zq8, trace=trace)
    if trace:
        kernel.last_results = res

    # --- host-side small terms ---
    # P_ must match what the device summed for the same-class entries, i.e.
    # the fp8-quantized sim values, so quantize the same way here.
    xq = zq8.astype(np.float64).T / S8   # [N, D] device-visible xn
    counts = np.bincount(yt, minlength=C)
    P_ = np.zeros(N, np.float64)
    slot0 = np.zeros(N, np.float64)
    for cval in np.unique(yt):
        idx = np.where(yt == cval)[0]
        subq = xq[idx]
        sq = (subq @ subq.T + 1.0) * (0.5 * TAU)
        P_[idx] = np.exp(sq).sum(1)
        if len(idx) >= 2:
            # slot0 feeds the final formula directly -> full precision
            sub = xn[idx].astype(np.float64)
            s = (sub @ sub.T + 1.0) * (0.5 * TAU)
            firstpos = np.where(np.arange(len(idx)) == 0, 1, 0)
            slot0[idx] = s[np.arange(len(idx)), firstpos]

    num_neg = N - counts[yt]
    S = T - P_
    Z = (2 * N - 2 - num_neg).astype(np.float64)
    cl = (np.log(np.exp(slot0) + S + Z) - slot0).mean()

    # cross-entropy in f64 on host (O(N*C))
    ypd = yp.astype(np.float64)
    mp = ypd.max(axis=1, keepdims=True)
    lse = np.log(np.exp(ypd - mp).sum(axis=1)) + mp[:, 0]
    ce = (lse - ypd[np.arange(N), yt]).mean()

    loss = LAMBD * cl + (1.0 - LAMBD) * ce
    return np.asarray(loss, dtype=np.float32)


# revision 14
# speedup vs baseline: 1.2902x; 1.1626x over previous
"""CLCE loss kernel for Trainium2 (8 NeuronCores, SPMD) — symmetric version.

Loss = 0.5 * cl + 0.5 * ce where
  cl_i = log(exp(slot0_i) + (T_i - P_i) + Z_i) - slot0_i
  T_i  = sum_j exp((xn_i . xn_j + 1) * 0.25)     <- O(N^2 D), on device
  P_i, slot0_i: same-class corrections, on host (exact, tiny)
  ce: cross-entropy of y_pred, on host in f64 (O(N*C), tiny)

Device: the N x N exp-sim row-sum exploits symmetry — only the upper
triangle of the 32x32 grid of 128-cells is computed (528 of 1024 cells).
Each computed strip contributes its row-sums via the Scalar engine's
accum_out, and its mirrored contribution via column-sums: exp tiles are
accumulated per column-tile on the Vector engine (bf16) and reduced
across partitions with one ones-matmul per column slot.

Uniform SPMD structure: every core runs the identical 9-item schedule
(diag staircase upper/lower + 7 off-diagonal 256-row half-blocks) over
6 data slots (4 column tiles + 2 flexible weight-row slots).  The host
chooses per-core slot contents so the 8 cores tile the triangle exactly:
  slots(c)  = [c, c+1, c+2, T3[c]]  (mod 8), T3 = [4,5,6,7,7,4,5,6]
  W halves  = difference-class patches (d2-h1, d3-h0, d3-h1, d4 halves)
Embeddings are fp8 (pre-scaled by S8) with DoubleRow matmuls, identical
quantization to what the host correction terms replicate.
"""

import os
from contextlib import ExitStack

import numpy as np

import concourse.bass as bass
import concourse.tile as tile
from concourse import bacc, mybir
from concourse.bass_utils import run_bass_kernel_spmd

N, D, C = 4096, 1024, 512
TAU = 0.5
LAMBD = 0.5
NCORES = 8
P = 128                    # partitions
KT = D // 256              # 4 DoubleRow contraction super-tiles
TW = 512                   # tile width (columns per tile slot)
S8 = 16.0                  # fp8 pre-scale for the embeddings
NSLOT = 6                  # 4 column tiles + 2 weight-row slots
NWARM = 6                  # PE warm-up matmuls

_F32 = mybir.dt.float32
_BF16 = mybir.dt.bfloat16
_FP8 = mybir.dt.float8e4
_EXP = mybir.ActivationFunctionType.Exp
_DR = mybir.MatmulPerfMode.DoubleRow

# ---------------- cover tables (validated exact) ----------------
T3 = [4, 5, 6, 7, 7, 4, 5, 6]


def _slots_of(c):
    return [c, (c + 1) % 8, (c + 2) % 8, T3[c]]


def _whalves_of(c):
    # (tile, half) for W halves 0..3 (slot4 m01, slot4 m23, slot5 m01, slot5 m23)
    return [((c - 2) % 8, 1), ((c - 2) % 8, 0), ((c - 1) % 8, 1),
            (T3[c] - 4, 0 if c < 4 else 1)]


# item: (row_slot, mlo, col_slot, kind); row_slot 0..3 = tile, 4..5 = W slots
ITEMS = [
    (0, 0, 0, 'DU'),   # I0 diag staircase rows m0,m1
    (0, 2, 0, 'DL'),   # I1 diag staircase rows m2,m3
    (0, 0, 1, 'OFF'),  # I2 (c -> c+1, h0)
    (1, 2, 2, 'OFF'),  # I3 (c+1 -> c+2, h1)
    (0, 0, 2, 'OFF'),  # I4 (c -> c+2, h0)
    (4, 0, 0, 'OFF'),  # I5 W patch, cols slot0
    (4, 2, 1, 'OFF'),  # I6 W patch, cols slot1
    (5, 0, 2, 'OFF'),  # I7 W patch, cols slot2
    (5, 2, 3, 'OFF'),  # I8 W patch, cols slot3
]
# device schedule order (I1 last: its final chunk has no colsum -> short tail)
ORDER = [0, 2, 5, 6, 3, 4, 7, 8, 1]


def _item_chunks(kind, mloc):
    """(mm_lo, mm_hi, cs_lo, cs_hi) column ranges for one m-group.
    [mm_lo, mm_hi) is computed+row-summed; [cs_lo, cs_hi) feeds colsum
    (strict-upper cells; diag 128-cells excluded).  cs_lo==cs_hi: none."""
    if kind == 'OFF':
        return (0, 512, 0, 512)
    if kind == 'DU':
        if mloc == 0:
            return (0, 512, 128, 512)
        return (128, 512, 256, 512)
    # DL
    if mloc == 0:
        return (256, 512, 384, 512)
    return (384, 512, 512, 512)


def _item_rows(c, item, mloc):
    """Global start row of the 128-row group (item, mloc) computes."""
    slot, mlo = item[0], item[1]
    if slot < 4:
        return _slots_of(c)[slot] * 512 + (mlo + mloc) * 128
    t, h = _whalves_of(c)[(slot - 4) * 2 + mlo // 2]
    return t * 512 + h * 256 + mloc * 128


def _emission_chunks():
    """(item_idx, mloc) in device emission order = accum col order."""
    return [(it, mloc) for it in ORDER for mloc in (0, 1)]


NRS = len(_emission_chunks())   # rowsum output columns (18)


# ---------------- device kernel ----------------
def _build_kernel(tc, xt, out1, out2):
    nc = tc.nc
    act_scale = 0.5 * TAU / (S8 * S8)
    with ExitStack() as ctx:
        pers = ctx.enter_context(tc.tile_pool(name="pers", bufs=1))
        epool = ctx.enter_context(tc.tile_pool(name="epool", bufs=3))
        psum = ctx.enter_context(
            tc.tile_pool(name="psum", bufs=4, space=bass.MemorySpace.PSUM)
        )
        cspsum = ctx.enter_context(
            tc.tile_pool(name="cspsum", bufs=2, space=bass.MemorySpace.PSUM)
        )
        cspsum0 = ctx.enter_context(
            tc.tile_pool(name="cspsum0", bufs=1, space=bass.MemorySpace.PSUM)
        )

        SLOT = [
            pers.tile([P, KT, 2, TW], _FP8, name=f"slot{s}", tag=f"slot{s}")
            for s in range(NSLOT)
        ]
        A = [pers.tile([P, TW], _BF16, name=f"acc{s}") for s in range(4)]
        OUTSB = pers.tile([P, NRS], _F32)
        CSSB = pers.tile([1, 4 * TW], _F32)
        bias_s = pers.tile([P, 1], _F32)
        bias_z = pers.tile([P, 1], _F32)
        warm = pers.tile([P, 1], _F32)
        ones = pers.tile([P, 1], _BF16)
        ZW = pers.tile([P, 512], _BF16)

        nc.gpsimd.memset(ZW[:], 0.0)
        nc.gpsimd.memset(bias_s[:], 0.5 * TAU)
        nc.gpsimd.memset(bias_z[:], 0.0)
        nc.gpsimd.memset(ones[:], 1.0)
        for s in range(4):
            nc.gpsimd.memset(A[s][:], 0.0)
        # warm the exp table before any data lands
        nc.scalar.activation(warm[:], bias_z[:], _EXP, bias=bias_z[:], scale=1.0)

        # PE warm-up: dummy matmuls spanning the input-DMA latency flip the
        # HAM clock gate to 8/8 so the real stream runs at 2.4GHz
        wps = psum.tile([P, 512], _F32, tag="ps")
        for _ in range(NWARM):
            nc.tensor.matmul(wps[:, 0:512], ZW[:, 0:P], ZW[:], start=True,
                             stop=True)

        # input DMAs: single HWDGE queue in exact first-use order so the
        # critical first bytes never share SDMA round-robin slots.  slot0 is
        # split per contraction super-step so I0's k=0 matmul starts after
        # 128KB instead of 512KB.
        xt6 = xt.rearrange("p (s k i n) -> p s k i n", s=NSLOT, k=KT, i=2)
        for k in range(KT):
            nc.sync.dma_start(SLOT[0][:, k], xt6[:, 0, k])
        for s in (1, 4, 2, 5, 3):
            nc.sync.dma_start(SLOT[s][:], xt6[:, s])

        # main schedule
        rs_col = 0
        # colsum reduction points (item_idx -> A slots), each one item after
        # its last DVE contribution so the ones-matmul never stalls the PE.
        # A0 closes after I5 (diag-lower chunk bypasses it via 'dl' below).
        cs_after = {6: [0], 3: [1], 8: [2], 1: [3, 'dl']}
        cs_tile = {}

        def colsum_reduce(s):
            if s == 'dl':
                # diag-lower strict-upper cell: accumulate into cs0's psum
                cps, et = cs_tile[0], cs_tile['dl_et']
                nc.tensor.matmul(cps[0:1, 384:512], ones[:, 0:1],
                                 et[:, 128:256], start=False, stop=True)
                nc.vector.tensor_copy(CSSB[0:1, 0:TW], cps[0:1, :])
                return
            pool = cspsum0 if s == 0 else cspsum
            cps = pool.tile([P, TW], _F32, tag="cs0" if s == 0 else "cs")
            nc.tensor.matmul(cps[0:1, :], ones[:, 0:1], A[s][:, :], start=True,
                             stop=(s != 0))
            if s == 0:
                cs_tile[0] = cps
            else:
                nc.vector.tensor_copy(CSSB[0:1, s * TW:(s + 1) * TW],
                                      cps[0:1, :])

        for it in ORDER:
            row_slot, mlo, col_slot, kind = ITEMS[it]
            for mloc in (0, 1):
                mm_lo, mm_hi, cs_lo, cs_hi = _item_chunks(kind, mloc)
                mcol = mlo + mloc
                ps = psum.tile([P, 512], _F32, tag="ps")
                for k in range(KT):
                    nc.tensor.matmul(
                        ps[:, mm_lo:mm_hi],
                        SLOT[row_slot][:, k, :, mcol * P:(mcol + 1) * P],
                        SLOT[col_slot][:, k, :, mm_lo:mm_hi],
                        start=(k == 0),
                        stop=(k == KT - 1),
                        perf_mode=_DR,
                    )
                et = epool.tile([P, 512], _BF16, tag="et")
                nc.scalar.activation(
                    et[:, 0:mm_hi - mm_lo], ps[:, mm_lo:mm_hi], _EXP,
                    bias=bias_s[:], scale=act_scale,
                    accum_out=OUTSB[:, rs_col:rs_col + 1],
                )
                rs_col += 1
                if cs_lo < cs_hi:
                    if kind == 'DL':
                        cs_tile['dl_et'] = et   # reduced directly at the end
                    else:
                        nc.vector.tensor_add(
                            A[col_slot][:, cs_lo:cs_hi],
                            A[col_slot][:, cs_lo:cs_hi],
                            et[:, cs_lo - mm_lo:cs_hi - mm_lo],
                        )
            for s in cs_after.get(it, ()):
                colsum_reduce(s)

        nc.scalar.dma_start(out1[:], OUTSB[:])
        nc.sync.dma_start(out2[:], CSSB[:])


_NC_CACHE = None


def _get_nc():
    global _NC_CACHE
    if _NC_CACHE is None:
        nc = bacc.Bacc(
            "TRN2", target_bir_lowering=False, debug=False,
            enable_asserts=False, num_devices=NCORES,
        )
        xt_d = nc.dram_tensor("xt", [P, NSLOT * KT * 2 * TW], _FP8,
                              kind="ExternalInput")
        out1_d = nc.dram_tensor("out1", [P, NRS], _F32, kind="ExternalOutput")
        out2_d = nc.dram_tensor("out2", [1, 4 * TW], _F32,
                                kind="ExternalOutput")
        with tile.TileContext(nc) as tc:
            _build_kernel(tc, xt_d.ap(), out1_d.ap(), out2_d.ap())
        nc.compile()
        _NC_CACHE = nc
    return _NC_CACHE


def _pack_cols(cols):
    """[D, 512] fp8 column block -> [P, 4096] with DoubleRow pairing:
    partition p, byte (k*2 + i)*512 + n  <->  contraction index
    k*256 + 128*i + p  of column n."""
    q = cols.reshape(KT, 2, P, TW).transpose(2, 0, 1, 3).reshape(P, KT * 2 * TW)
    return np.ascontiguousarray(q)


def _run_device(zq8, trace=False):
    """zq8: [D, N] fp8 pre-scaled quantized embeddings (as fp8 np dtype)."""
    in_maps = []
    for c in range(NCORES):
        slots = _slots_of(c)
        wh = _whalves_of(c)
        parts = []
        for s in range(4):
            t = slots[s]
            parts.append(_pack_cols(zq8[:, t * 512:(t + 1) * 512]))
        for w0 in (0, 2):  # slots 4, 5
            blk = np.concatenate(
                [zq8[:, wh[w0 + j][0] * 512 + wh[w0 + j][1] * 256:][:, :256]
                 for j in range(2)], axis=1)
            parts.append(_pack_cols(np.ascontiguousarray(blk)))
        in_maps.append({"xt": np.concatenate(parts, axis=1)})
    res = run_bass_kernel_spmd(
        _get_nc(), in_maps, core_ids=list(range(NCORES)), trace=trace,
    )
    T = np.zeros(N, np.float64)
    chunks = _emission_chunks()
    for c, r in enumerate(res.results):
        o1 = r["out1"].astype(np.float64)   # [P, NRS]
        o2 = r["out2"].astype(np.float64)   # [1, 4*TW]
        slots = _slots_of(c)
        for col, (it, mloc) in enumerate(chunks):
            r0 = _item_rows(c, ITEMS[it], mloc)
            T[r0:r0 + P] += o1[:, col]
        for s in range(4):
            t = slots[s]
            T[t * 512:(t + 1) * 512] += o2[0, s * TW:(s + 1) * TW]
    return T, res


def kernel(layer_embeds, y_true, y_pred):
    x = np.asarray(layer_embeds, dtype=np.float32)
    yt = np.asarray(y_true).astype(np.int64)
    yp = np.asarray(y_pred, dtype=np.float32)

    # normalize rows (torch-style eps clip)
    norms = np.maximum(
        np.sqrt((x.astype(np.float64) ** 2).sum(1, keepdims=True)), 1e-8
    )
    xn = (x / norms).astype(np.float32)
    fp8np = mybir.dt.np(_FP8)
    zq8 = np.ascontiguousarray((xn.T * S8).astype(np.float32)).astype(fp8np)

    trace = bool(int(os.environ.get("CLCE_TRACE", "0")))
    T, res = _run_device(zq8, trace=trace)
    if trace:
        kernel.last_results = res

    # --- host-side small terms ---
    # P_ must match what the device summed for the same-class entries, i.e.
    # the fp8-quantized sim values, so quantize the same way here.
    xq = zq8.astype(np.float64).T / S8   # [N, D] device-visible xn
    counts = np.bincount(yt, minlength=C)
    P_ = np.zeros(N, np.float64)
    slot0 = np.zeros(N, np.float64)
    for cval in np.unique(yt):
        idx = np.where(yt == cval)[0]
        subq = xq[idx]
        sq = (subq @ subq.T + 1.0) * (0.5 * TAU)
        P_[idx] = np.exp(sq).sum(1)
        if len(idx) >= 2:
            # slot0 feeds the final formula directly -> full precision
            sub = xn[idx].astype(np.float64)
            s = (sub @ sub.T + 1.0) * (0.5 * TAU)
            firstpos = np.where(np.arange(len(idx)) == 0, 1, 0)
            slot0[idx] = s[np.arange(len(idx)), firstpos]

    num_neg = N - counts[yt]
    S = T - P_
    Z = (2 * N - 2 - num_neg).astype(np.float64)
    cl = (np.log(np.exp(slot0) + S + Z) - slot0).mean()

    # cross-entropy in f64 on host (O(N*C))
    ypd = yp.astype(np.float64)
    mp = ypd.max(axis=1, keepdims=True)
    lse = np.log(np.exp(ypd - mp).sum(axis=1)) + mp[:, 0]
    ce = (lse - ypd[np.arange(N), yt]).mean()

    loss = LAMBD * cl + (1.0 - LAMBD) * ce
    return np.asarray(loss, dtype=np.float32)
